# revision 17
# baseline (speedup 1.0000x reference)
# Cross-attention kernel for Trainium2, 8 NeuronCores.
#
# Sharding: data-parallel over (batch, query-half): core = 2*b + half handles
# batch b, queries [half*1024, (half+1)*1024). No collectives.
#
# On-device layout is feature-major: activations live as [feature, token] in
# fp16. Both layernorms fold into the projections via the postscale form
#   LN(x) @ W'.T = (x @ W' + [-S; bq] x [m; std]) * rstd,
# so the PSUM->SBUF evacuation copy becomes the rstd multiply. The finale
# keeps the prescale form so Gelu reads its PSUM accumulator directly.
#
# v18 structure:
# - All matmul operands are fp16 (fp32r runs in fp32_mode=HIGH at ~1.5
#   cyc/col on HW; fp16 is 1). The finale weights/stats and the K/Q aux rows
#   are fp16; only PSUM accumulation and the DVE stat chains stay fp32.
# - Attention units are c-major so the c=0 finale streams inside the exp
#   window. V and the later K/Q projection chunks stream as PE tasks.
# - Finale LN stats accumulate per-jc into SBUF as each (c,P) column block
#   finishes, so the tail only runs the last jc + Wo chain.
# - All LN stats run K-first in the prefix (kt arrives first) so the ACT
#   engine never reloads the Exp table mid-stream; each finale block costs
#   one Sqrt+Gelu table excursion.
# - DMA: sync queue carries the bulk (K, V, finale weights); scalar triggers
#   only eps/qt/wk/wq/wv then frees ACT for the stats chain; gpsimd stays
#   near-empty for latency-critical rows (aux stats, den rows, O evac, out).
# - Output fp16 (absmax ~5, quantization ~3e-4 abs).
import os
import sys
import tempfile

os.environ["NEURON_COMPILE_CACHE_URL"] = tempfile.mkdtemp(prefix="neff_cache_")
os.environ["AXON_CASSETTE_SALT"] = f"ca-{os.getpid()}-{os.urandom(4).hex()}"

for _p in ("/opt/trn_rl_repo",):
    if os.path.isdir(_p) and _p not in sys.path:
        sys.path.insert(0, _p)

import numpy as np
from contextlib import ExitStack

import concourse.bass as bass
import concourse.tile as tile
from concourse import bacc, mybir
from concourse.bass_utils import run_bass_kernel_spmd

F32 = mybir.dt.float32
F16 = mybir.dt.float16
AF = mybir.ActivationFunctionType
ALU = mybir.AluOpType

B, NQ, NK, D = 4, 2048, 2048, 512
H, DH = 8, 64
NQS = NQ // 2  # queries per core
TEMP = float(np.sqrt(512.0))
LN_EPS = 1e-5
N_CORES = 8

_CACHE = {}
SALT = "v19"
SALT_N = 19


def _build_program():
    nc = bacc.Bacc("TRN2", target_bir_lowering=False, debug=False)

    def din(name, shape, dt=F16):
        return nc.dram_tensor(f"{name}_{SALT}", shape, dt,
                              kind="ExternalInput").ap()

    qt_d = din("qt", [128, 4, NQS])
    kt_d = din("kt", [128, 4, NK])
    vt_d = din("vt", [128, 4, NK])
    wq_d = din("wq", [128, 4, D])
    wk_d = din("wk", [128, 4, D])
    wv_d = din("wv", [128, 4, D])
    wo_d = din("wo", [128, 4, D])
    aq_d = din("aq", [2, D])
    ak_d = din("ak", [2, D])
    ao_d = din("ao", [2, D])
    gb_d = din("gb", [128, 2, 4], F32)
    grow_d = din("grow", [1, D])
    salt_d = din("salt", [1, 8 + SALT_N], F32)
    out_d = nc.dram_tensor(f"out_{SALT}", [128, 4, NQS], F16,
                           kind="ExternalOutput").ap()

    with tile.TileContext(nc) as tc, ExitStack() as top:
        persist = top.enter_context(tc.tile_pool(name="persist", bufs=1))
        qTs = persist.tile([128, 4, NQS], F16)        # projected q
        kTs = persist.tile([128, 4, NK], F16)         # projected k
        vaug = persist.tile([128, 16, 8, 65], F16)    # v natural + ones col
        oTs = persist.tile([128, 4, NQS], F16)        # attention out
        qtin = persist.tile([128, 4, NQS], F16)
        ktin = persist.tile([128, 4, NK], F16)
        vtin = persist.tile([128, 4, NK], F16)
        wq_sb = persist.tile([128, 4, D], F16)
        wk_sb = persist.tile([128, 4, D], F16)
        wv_sb = persist.tile([128, 4, D], F16)
        wo_sb = persist.tile([128, 4, D], F16)
        aq_sb = persist.tile([2, D], F16)
        ak_sb = persist.tile([2, D], F16)
        ao_sb = persist.tile([2, D], F16)
        gb_sb = persist.tile([128, 2, 4], F32)
        grow_sb = persist.tile([1, D], F16)
        auxq = persist.tile([2, NQS], F16)            # [m; std] rows for Q
        auxk = persist.tile([2, NK], F16)
        auxo = persist.tile([2, NQS], F16)            # [m*r; 1] rows, finale
        rstdQ = persist.tile([128, 2, 512], F32)      # replicated 1/std per
        rstdK = persist.tile([128, 4, 512], F32)      # 512-token chunk
        fsum = persist.tile([128, 2, 512], F32)       # finale sum accum
        fssq = persist.tile([128, 2, 512], F32)       # finale ssq accum
        onesh = persist.tile([128, 128], F16)
        eps_t = persist.tile([128, 1], F32)

        # DMA queues (only SP/ACT/gpsimd trigger): sync = bulk, scalar =
        # critical small loads only (a 5th+ trigger waits on semaphore
        # recycling, pinning its engine), gpsimd = near-empty for
        # latency-critical mid-stream rows.
        nc.scalar.dma_start(out=eps_t,
                            in_=salt_d[0:1, 0:1].to_broadcast([128, 1]))
        nc.scalar.dma_start(out=wk_sb, in_=wk_d)
        for n2 in range(2):
            ns = slice(512 * n2, 512 * n2 + 512)
            nc.scalar.dma_start(out=qtin[:, :, ns], in_=qt_d[:, :, ns])
        for n2 in range(4):
            ns = slice(512 * n2, 512 * n2 + 512)
            nc.sync.dma_start(out=ktin[:, :, ns], in_=kt_d[:, :, ns])
        nc.scalar.dma_start(out=wq_sb, in_=wq_d)
        nc.gpsimd.dma_start(out=aq_sb, in_=aq_d)
        nc.gpsimd.dma_start(out=ak_sb, in_=ak_d)
        nc.scalar.dma_start(out=wv_sb, in_=wv_d)
        for n2 in range(4):
            ns = slice(512 * n2, 512 * n2 + 512)
            nc.sync.dma_start(out=vtin[:, :, ns], in_=vt_d[:, :, ns])
        nc.sync.dma_start(out=gb_sb, in_=gb_d)
        nc.sync.dma_start(out=grow_sb, in_=grow_d)
        nc.sync.dma_start(out=ao_sb, in_=ao_d)
        nc.sync.dma_start(out=wo_sb, in_=wo_d)
        nc.vector.memset(onesh, 1.0)
        nc.vector.memset(vaug[:, :, :, 64], 1.0)
        # row 0 is overwritten with m*r in the finale; row 1 stays all-ones
        nc.vector.memset(auxo, 1.0)
        nc.vector.memset(fsum, 0.0)
        nc.vector.memset(fssq, 0.0)

        pmm = top.enter_context(tc.tile_pool(name="pmm", bufs=1, space="PSUM"))
        work = top.enter_context(tc.tile_pool(name="work", bufs=1))
        # Prefix-only stats accumulators, released before the attention
        # pools open.
        pre_ctx = tc.tile_pool(name="pre", bufs=1, space="PSUM")
        pre = pre_ctx.__enter__()

        def ln_stats(xin, n2, aux, rstd_store, pool):
            """Column LN stats of token chunk n2 of xin [128, 4, *]; fills
            aux rows [m; std] (fp16) and rstd_store[:, n2, :]."""
            ns = slice(512 * n2, 512 * n2 + 512)
            ps_sum = pool.tile([128, 512], F32, name="st", bufs=4)
            for kc in range(4):
                nc.tensor.matmul(ps_sum, onesh, xin[:, kc, ns],
                                 start=(kc == 0), stop=(kc == 3))
            ps_ssq = pool.tile([128, 512], F32, name="st", bufs=4)
            for kc in range(4):
                sq = work.tile([128, 512], F16, name="sq", bufs=3)
                with nc.allow_low_precision("squares in fp16"):
                    nc.vector.tensor_mul(sq, xin[:, kc, ns], xin[:, kc, ns])
                nc.tensor.matmul(ps_ssq, onesh, sq,
                                 start=(kc == 0), stop=(kc == 3))
            m_b = work.tile([128, 512], F16, name="m16", bufs=3)
            with nc.allow_low_precision("fp16 mean"):
                nc.scalar.mul(m_b, ps_sum, 1.0 / 512.0)
            t2 = work.tile([128, 512], F32, name="w32", bufs=6)
            nc.vector.tensor_mul(t2, m_b, ps_sum)
            dv = work.tile([128, 512], F32, name="w32", bufs=6)
            nc.vector.tensor_sub(dv, ps_ssq, t2)
            std_b = work.tile([128, 512], F32, name="w32", bufs=6)
            nc.scalar.activation(std_b, dv, AF.Sqrt, bias=eps_t,
                                 scale=1.0 / 512.0)
            nc.vector.reciprocal_approx_fast(rstd_store[:, n2, :], std_b)
            s16 = work.tile([1, 512], F16, name="s16", bufs=3)
            with nc.allow_low_precision("fp16 std row"):
                nc.vector.tensor_copy(s16, std_b[0:1, :])
            # engine ops cannot move data across partitions; DMA the stat
            # rows into the K=2 aux operand (gpsimd queue: empty, fast).
            nc.gpsimd.dma_start(out=aux[0:1, ns], in_=m_b[0:1, :])
            nc.gpsimd.dma_start(out=aux[1:2, ns], in_=s16)

        def proj_chunk(dst, xin, w_sb, a_sb, aux, rstd_store, jc, n2):
            """dst[:, jc, ns] = (sum_kc w'[:,kc,js].T @ x[:,kc,ns] + aux)*r."""
            ns = slice(512 * n2, 512 * n2 + 512)
            js = slice(128 * jc, 128 * jc + 128)
            pg = pmm.tile([128, 512], F32, name="pmm", bufs=2)
            for kc in range(4):
                nc.tensor.matmul(pg, w_sb[:, kc, js], xin[:, kc, ns],
                                 start=(kc == 0), stop=False)
            nc.tensor.matmul(pg, a_sb[:, js], aux[:, ns],
                             start=False, stop=True)
            with nc.allow_low_precision("fp16 activations"):
                nc.vector.tensor_mul(dst[:, jc, ns], pg, rstd_store[:, n2, :])

        def v_chunk(t):
            ts = slice(128 * t, 128 * t + 128)
            pv = pmm.tile([128, 512], F32, name="pmm", bufs=2)
            for kc in range(4):
                nc.tensor.matmul(pv, vtin[:, kc, ts], wv_sb[:, kc, :],
                                 start=(kc == 0), stop=(kc == 3))
            with nc.allow_low_precision("fp16 activations"):
                nc.vector.tensor_copy(
                    vaug[:, t, :, 0:64],
                    pv.rearrange("p (h v) -> p h v", h=8))

        # ---- Prefix: all LN stats (K first: kt lands first) so ACT never
        # swaps tables mid-exp; then only the two unit-0-gating projections.
        # Everything else streams through the window. ----
        for n2 in range(4):
            ln_stats(ktin, n2, auxk, rstdK, pre)
        ln_stats(qtin, 0, auxq, rstdQ, pre)
        ln_stats(qtin, 1, auxq, rstdQ, pre)
        proj_chunk(kTs, ktin, wk_sb, ak_sb, auxk, rstdK, 0, 0)
        proj_chunk(qTs, qtin, wq_sb, aq_sb, auxq, rstdQ, 0, 0)
        pre_ctx.__exit__(None, None, None)  # release stats banks for psL

        # Streaming tasks: remaining projections and the finale stats run
        # through the attention window under the ACT exp stream.
        tasks = {}

        def add_task(u, fn):
            tasks.setdefault(u, []).append(fn)

        def mk_proj(dst, xin, w_sb, a_sb, aux, rstd_store, jc, n2):
            return lambda: proj_chunk(dst, xin, w_sb, a_sb, aux, rstd_store,
                                      jc, n2)

        for t in range(16):
            add_task(t, lambda t=t: v_chunk(t))
        # K proj (jc=P, n2) due by unit 16*P + 4*n2 - 1; Q proj (jc, n2=c)
        # due by unit 64*c + 16*jc - 1. V chunk t due by unit t + LAG - 1.
        kslot = {(0, 1): 0, (0, 2): 4, (0, 3): 8,
                 (1, 0): 12, (1, 1): 14, (1, 2): 18, (1, 3): 22,
                 (2, 0): 26, (2, 1): 30, (2, 2): 34, (2, 3): 38,
                 (3, 0): 42, (3, 1): 46, (3, 2): 50, (3, 3): 54}
        for (jc, n2), u in kslot.items():
            add_task(u, mk_proj(kTs, ktin, wk_sb, ak_sb, auxk, rstdK, jc, n2))
        qslot = {(1, 0): 10, (2, 0): 24, (3, 0): 40,
                 (0, 1): 58, (1, 1): 60, (2, 1): 62, (3, 1): 64}
        for (jc, n2), u in qslot.items():
            add_task(u, mk_proj(qTs, qtin, wq_sb, aq_sb, auxq, rstdQ, jc, n2))

        # ---- Attention: units (c, P, p); c-major so the c=0 finale can
        # stream inside the window. ----
        at_psL = top.enter_context(tc.tile_pool(name="at_psL", bufs=1,
                                                space="PSUM"))
        at_po = top.enter_context(tc.tile_pool(name="at_po", bufs=1,
                                               space="PSUM"))
        at_sb = top.enter_context(tc.tile_pool(name="at_sb", bufs=1))

        units = [(c, P, p) for c in range(2) for P in range(4)
                 for p in range(16)]
        LAG = 4
        pend = {}   # unit idx -> (P, c, p, psO pair, ex tile)

        def emit_O(u):
            P, c, p, ps_o, ex = pend.pop(u)
            for hh in range(2):
                nc.tensor.matmul(ps_o[hh], vaug[:, p, 2 * P + hh, :],
                                 ex[:, 512 * hh: 512 * hh + 512],
                                 start=(p == 0), stop=(p == 15))
            if p == 15:
                cs = slice(512 * c, 512 * c + 512)
                for hh in range(2):
                    # One quick DVE copy evacuates the accumulator bank so
                    # the next (P,c) group's first O-matmul isn't blocked
                    # behind the normalize chain.
                    oc = at_sb.tile([65, 512], F32, name="oc", bufs=2)
                    nc.vector.tensor_copy(oc, ps_o[hh])
                    # recip_approx works from SBUF partition 0: DMA the den
                    # row down, then reciprocal + broadcast.
                    den0 = at_sb.tile([1, 512], F32, name="tl32", bufs=6)
                    nc.gpsimd.dma_start(out=den0, in_=oc[64:65, :])
                    rr0 = at_sb.tile([1, 512], F32, name="tl32", bufs=6)
                    nc.vector.reciprocal_approx_fast(rr0, den0)
                    rrb = at_sb.tile([64, 512], F32, name="tl32", bufs=6)
                    nc.gpsimd.partition_broadcast(rrb, rr0)
                    rb = 64 * hh
                    if hh == 0:
                        # rows align with oTs: write the normalized block
                        # in place, no DMA hop.
                        with nc.allow_low_precision("fp16 storage"):
                            nc.vector.tensor_mul(oTs[0:64, P, cs],
                                                 oc[0:64, :], rrb)
                    else:
                        ost = at_sb.tile([64, 512], F16, name="ost", bufs=2)
                        with nc.allow_low_precision("fp16 storage"):
                            nc.vector.tensor_mul(ost, oc[0:64, :], rrb)
                        # sync queue: gpsimd's hw queue is shallow and a
                        # 64KB transfer there stalls the den-row DMAs of
                        # later groups (PE then waits and drops p-state).
                        nc.sync.dma_start(out=oTs[rb:rb + 64, P, cs],
                                          in_=ost)

        # ---- Finale: LN fold (prescale form) + Wo + gelu + residual.
        # Stats accumulate per-jc into SBUF as each (c, P=jc) column block
        # completes; the Wo chain runs as one clustered block per 512-token
        # chunk (its ACT ops sit consecutively: one Sqrt+Gelu excursion). ----
        def fin_stat(jc, n2):
            ns = slice(512 * n2, 512 * n2 + 512)
            pg = pmm.tile([128, 512], F32, name="pmm", bufs=2)
            nc.tensor.matmul(pg, onesh, oTs[:, jc, ns], start=True, stop=True)
            nc.vector.tensor_add(fsum[:, n2, :], fsum[:, n2, :], pg)
            sqo = work.tile([128, 512], F16, name="sq", bufs=3)
            with nc.allow_low_precision("fp16 squares"):
                nc.vector.tensor_mul(sqo, oTs[:, jc, ns], oTs[:, jc, ns])
            pq = pmm.tile([128, 512], F32, name="pmm", bufs=2)
            nc.tensor.matmul(pq, onesh, sqo, start=True, stop=True)
            nc.vector.tensor_add(fssq[:, n2, :], fssq[:, n2, :], pq)

        def fin_wo(n2):
            ns = slice(512 * n2, 512 * n2 + 512)
            m_b = work.tile([128, 512], F32, name="w32", bufs=6)
            nc.scalar.mul(m_b, fsum[:, n2, :], 1.0 / 512.0)
            t2 = work.tile([128, 512], F32, name="w32", bufs=6)
            nc.vector.tensor_mul(t2, m_b, fsum[:, n2, :])
            dv = work.tile([128, 512], F32, name="w32", bufs=6)
            nc.vector.tensor_sub(dv, fssq[:, n2, :], t2)
            std_b = work.tile([128, 512], F32, name="w32", bufs=6)
            nc.scalar.activation(std_b, dv, AF.Sqrt, bias=eps_t,
                                 scale=1.0 / 512.0)
            r_b = work.tile([128, 512], F32, name="w32", bufs=6)
            nc.vector.reciprocal_approx_fast(r_b, std_b)
            with nc.allow_low_precision("fp16 storage"):
                nc.vector.tensor_mul(auxo[0:1, ns], m_b[0:1, :], r_b[0:1, :])
                for jc in range(4):
                    nc.vector.tensor_mul(oTs[:, jc, ns], oTs[:, jc, ns], r_b)
            for jc in range(4):
                js = slice(128 * jc, 128 * jc + 128)
                pg = pmm.tile([128, 512], F32, name="pmm", bufs=2)
                for kc in range(4):
                    nc.tensor.matmul(pg, wo_sb[:, kc, js], oTs[:, kc, ns],
                                     start=(kc == 0), stop=False)
                nc.tensor.matmul(pg, ao_sb[:, js], auxo[:, ns],
                                 start=False, stop=True)
                pbm = pmm.tile([128, 512], F32, name="pmm", bufs=2)
                nc.tensor.matmul(pbm, grow_sb[0:1, js], auxo[0:1, ns],
                                 start=True, stop=True)
                gl = work.tile([128, 512], F32, name="w32", bufs=6)
                nc.scalar.activation(gl, pg, AF.Gelu)
                u2 = work.tile([128, 512], F32, name="w32", bufs=6)
                # u2 = oTs_scaled*g - m*r*g   (oTs already prescaled by r)
                nc.vector.scalar_tensor_tensor(
                    u2, oTs[:, jc, ns], gb_sb[:, 0, jc:jc + 1], pbm,
                    op0=ALU.mult, op1=ALU.subtract)
                of = work.tile([128, 512], F16, name="wof", bufs=4)
                with nc.allow_low_precision("fp16 output"):
                    nc.vector.scalar_tensor_tensor(
                        of, u2, gb_sb[:, 1, jc:jc + 1], gl,
                        op0=ALU.add, op1=ALU.add)
                nc.sync.dma_start(out=out_d[:, jc, ns], in_=of)

        # (c, P=jc) column block of chunk n2=c completes at unit
        # 64*c + 16*jc + 15 (emitted LAG later; its normalize chain takes
        # another ~8 units of wall time — don't gate the PE on it early).
        for jc in range(4):
            add_task(16 * jc + 27, lambda jc=jc: fin_stat(jc, 0))
        add_task(84, lambda: fin_wo(0))
        for jc in range(3):
            add_task(64 + 16 * jc + 27, lambda jc=jc: fin_stat(jc, 1))

        ps_o_cur = None
        for u, (c, P, p) in enumerate(units):
            if p == 0:
                ps_o_cur = [at_po.tile([65, 512], F32, name=f"po{hh}",
                                       bufs=1) for hh in range(2)]
            psL = at_psL.tile([128, 1024], F32, name="psL", bufs=2)
            ks = slice(128 * p, 128 * p + 128)
            cs = slice(512 * c, 512 * c + 512)
            for hh in range(2):
                rb = 64 * hh
                nc.tensor.matmul(psL[:, 512 * hh: 512 * hh + 512],
                                 kTs[rb:rb + 64, P, ks],
                                 qTs[rb:rb + 64, P, cs],
                                 start=True, stop=True)
            ex = at_sb.tile([128, 1024], F16, name="ex", bufs=LAG + 1)
            nc.scalar.activation(ex, psL, AF.Exp, scale=1.0 / TEMP)
            pend[u] = (P, c, p, ps_o_cur, ex)
            if u >= LAG:
                emit_O(u - LAG)
            for fn in tasks.pop(u, ()):
                fn()
        for u in range(len(units) - LAG, len(units)):
            emit_O(u)

        fin_stat(3, 1)
        fin_wo(1)

    nc.compile()
    return nc


def _chunk_fm(x):
    """[512, N] feature-major -> [128, 4, N] (partition, chunk, col)."""
    n = x.shape[1]
    return np.ascontiguousarray(x.reshape(4, 128, n).transpose(1, 0, 2))


def _prep_inputs(Q, K, V, Wq, Wk, Wv, Wo, g, b, go, bo):
    WqT = np.ascontiguousarray((Wq * g[None, :]).T)
    WkT = np.ascontiguousarray((Wk * g[None, :]).T)
    WvT = np.ascontiguousarray(Wv.T)
    WoT = np.ascontiguousarray((Wo * go[None, :]).T)
    f16 = np.float16
    shared = {
        f"wq_{SALT}": _chunk_fm(WqT).astype(f16),
        f"wk_{SALT}": _chunk_fm(WkT).astype(f16),
        f"wv_{SALT}": _chunk_fm(WvT).astype(f16),
        f"wo_{SALT}": _chunk_fm(WoT).astype(f16),
        f"aq_{SALT}": np.stack([-WqT.sum(0), Wq @ b]).astype(f16),
        f"ak_{SALT}": np.stack([-WkT.sum(0), Wk @ b]).astype(f16),
        f"ao_{SALT}": np.stack([-WoT.sum(0), Wo @ bo]).astype(f16),
        f"gb_{SALT}": np.ascontiguousarray(
            np.stack([go.reshape(4, 128).T, bo.reshape(4, 128).T], axis=1)),
        f"grow_{SALT}": go[None, :].astype(f16),
    }
    in_maps = []
    for core in range(N_CORES):
        bi, half = core // 2, core % 2
        qs = slice(half * NQS, (half + 1) * NQS)
        m = dict(shared)
        m[f"salt_{SALT}"] = np.full((1, 8 + SALT_N), LN_EPS, np.float32)
        m[f"qt_{SALT}"] = _chunk_fm(np.ascontiguousarray(Q[bi, qs, :].T)).astype(f16)
        m[f"kt_{SALT}"] = _chunk_fm(np.ascontiguousarray(K[bi].T)).astype(f16)
        m[f"vt_{SALT}"] = _chunk_fm(np.ascontiguousarray(V[bi].T)).astype(f16)
        in_maps.append(m)
    return in_maps


def kernel(Q, K, V, Wq, Wk, Wv, Wo, ln_qk_g, ln_qk_b, ln_o_g, ln_o_b,
           _trace=False):
    args = [np.asarray(a, dtype=np.float32) for a in
            (Q, K, V, Wq, Wk, Wv, Wo, ln_qk_g, ln_qk_b, ln_o_g, ln_o_b)]
    if "nc" not in _CACHE:
        _CACHE["nc"] = _build_program()
    nc = _CACHE["nc"]
    in_maps = _prep_inputs(*args)
    res = run_bass_kernel_spmd(nc, in_maps, core_ids=list(range(N_CORES)),
                               trace=_trace)
    _CACHE["last_results"] = res
    out = np.empty((B, NQ, D), dtype=np.float32)
    for core in range(N_CORES):
        bi, half = core // 2, core % 2
        o = res.results[core][f"out_{SALT}"].astype(np.float32)  # [128,4,NQS]
        out[bi, half * NQS : (half + 1) * NQS, :] = (
            o.transpose(1, 0, 2).reshape(D, NQS).T)
    return out
